# revision 3
# baseline (speedup 1.0000x reference)
"""Causal ReLU-attention block (qkv proj + per-head attention) on 8 trn2 cores.

Sharding: pure data-parallel over batch (B=8 -> 1 batch element per core).
Per-core: x_b [T,C] -> qkv -> scoresT = k q^T (row-tiled head pairs) ->
relu/causal-mask -> yT = v^T-ish (col-tiled head pairs) -> DRAM yT [C,T].
Host side: transpose/cast shards in, transpose gather out.
"""

import sys
from contextlib import ExitStack

sys.path.insert(0, "/opt/trn_rl_repo")

import ml_dtypes
import numpy as np

import concourse.bass as bass
import concourse.tile as tile
from concourse import bacc, bass_utils, mybir

P = 128
QW = 512  # t_q chunk width (PSUM bank = 512 fp32)

BF16 = mybir.dt.bfloat16
F32 = mybir.dt.float32
AF = mybir.ActivationFunctionType
ALU = mybir.AluOpType


def build_module(T=1024, C=768, H=12, n_cores=8):
    """Build + compile the per-core Bass module (same program on all cores)."""
    hd = C // H
    assert hd == 64 and H % 2 == 0 and C % P == 0 and T % QW == 0
    CT = C // P            # contraction tiles over C
    OT = 2 * C // P        # o-tiles of the stacked [q; k] projection
    TT = T // P            # t tiles
    NQC = T // QW          # q chunks
    VW = C // 2            # v output free-dim chunk width (<=512)
    scale = 1.0 / float(np.sqrt(hd))

    nc = bacc.Bacc("TRN2", target_bir_lowering=False, debug=False,
                   num_devices=n_cores)

    xT = nc.dram_tensor("xT", [C, T], BF16, kind="ExternalInput").ap()
    wT = nc.dram_tensor("wT", [C, 3 * C], BF16, kind="ExternalInput").ap()
    bqk = nc.dram_tensor("bqk", [P, OT], F32, kind="ExternalInput").ap()
    bv = nc.dram_tensor("bv", [P, C], F32, kind="ExternalInput").ap()
    yT = nc.dram_tensor("yT", [C, T], F32, kind="ExternalOutput").ap()

    with tile.TileContext(nc) as tc, ExitStack() as ctx:
        const = ctx.enter_context(tc.tile_pool(name="const", bufs=1))
        psum = ctx.enter_context(tc.tile_pool(name="psum", bufs=4, space="PSUM"))
        ypsum = ctx.enter_context(tc.tile_pool(name="ypsum", bufs=4, space="PSUM"))
        scb = ctx.enter_context(tc.tile_pool(name="scb", bufs=6))
        ysb = ctx.enter_context(tc.tile_pool(name="ysb", bufs=3))

        wt_sb = const.tile([P, CT, 3 * C], BF16)
        nc.sync.dma_start(wt_sb[:], wT.rearrange("(ct p) o -> p ct o", p=P))
        xt_sb = const.tile([P, CT, T], BF16)
        nc.sync.dma_start(xt_sb[:], xT.rearrange("(ct p) t -> p ct t", p=P))
        bqk_sb = const.tile([P, OT], F32)
        nc.sync.dma_start(bqk_sb[:], bqk[:])
        bv_sb = const.tile([P, C], F32)
        nc.sync.dma_start(bv_sb[:], bv[:])

        qkT = const.tile([P, OT, T], BF16)       # o-tiles: q = 0..OT/2-1, k = rest
        vsb = const.tile([P, TT, C], BF16)       # v in natural [t, o] layout

        # ---- qkv projection ----
        # qkT[o, t]: lhsT = wT c-tile x o-slice, rhs = xT c-tile x t-chunk
        for ot in range(OT):
            ps = [psum.tile([P, QW], F32, tag="blk", name=f"qk_ps{i}") for i in range(NQC)]
            for ct in range(CT):
                for qc in range(NQC):
                    nc.tensor.matmul(
                        ps[qc][:],
                        wt_sb[:, ct, ot * P:(ot + 1) * P],
                        xt_sb[:, ct, qc * QW:(qc + 1) * QW],
                        start=(ct == 0), stop=(ct == CT - 1),
                    )
            for qc in range(NQC):
                nc.scalar.activation(
                    qkT[:, ot, qc * QW:(qc + 1) * QW], ps[qc][:],
                    AF.Identity, bias=bqk_sb[:, ot:ot + 1],
                )
        # v[t, o]: lhsT = xT c-tile x t-slice, rhs = wT c-tile x v-chunk
        for tt in range(TT):
            ps = [psum.tile([P, QW], F32, tag="blk", name=f"v_ps{i}") for i in range(2)]
            for ct in range(CT):
                for oc in range(2):
                    nc.tensor.matmul(
                        ps[oc][:, :VW],
                        xt_sb[:, ct, tt * P:(tt + 1) * P],
                        wt_sb[:, ct, 2 * C + oc * VW:2 * C + (oc + 1) * VW],
                        start=(ct == 0), stop=(ct == CT - 1),
                    )
            for oc in range(2):
                nc.vector.tensor_tensor(
                    vsb[:, tt, oc * VW:(oc + 1) * VW], ps[oc][:, :VW],
                    bv_sb[:, oc * VW:(oc + 1) * VW], ALU.add,
                )

        # ---- attention, one head pair (2*hd = 128 partitions) at a time ----
        kofs = OT // 2   # first k o-tile
        evict = 0
        for hp in range(H // 2):
            for qc in range(NQC):
                ypA = ypsum.tile([P, QW], F32, tag="y")   # head A -> [0:64]
                ypB = ypsum.tile([P, QW], F32, tag="y")   # head B -> [64:128]
                kb_hi = min((qc * QW + QW - 1) // P, TT - 1)
                for kb in range(kb_hi + 1):
                    spA = psum.tile([P, QW], F32, tag="blk")
                    spB = psum.tile([P, QW], F32, tag="blk")
                    # scoresT[t_k, t_q] = k^T.T @ q^T, row-tiled pair
                    nc.tensor.matmul(
                        spA[:], qkT[0:64, kofs + hp, kb * P:(kb + 1) * P],
                        qkT[0:64, hp, qc * QW:(qc + 1) * QW],
                        start=True, stop=True, tile_position=(0, 0),
                    )
                    nc.tensor.matmul(
                        spB[:], qkT[64:128, kofs + hp, kb * P:(kb + 1) * P],
                        qkT[64:128, hp, qc * QW:(qc + 1) * QW],
                        start=True, stop=True, tile_position=(64, 0),
                    )
                    sA = scb.tile([P, QW], BF16, tag="s")
                    sB = scb.tile([P, QW], BF16, tag="s")
                    # relu(scale * s), PSUM -> SBUF, alternating ACT / DVE
                    for sp, s in ((spA, sA), (spB, sB)):
                        if evict % 3 != 2:
                            nc.scalar.activation(s[:], sp[:], AF.Relu, scale=scale)
                        else:
                            nc.vector.tensor_scalar(
                                s[:], sp[:], scale, 0.0, ALU.mult, ALU.max)
                        evict += 1
                    # causal mask on diagonal blocks: keep where t_k <= t_q
                    delta = kb * P - qc * QW
                    if delta > -P:
                        for s in (sA, sB):
                            nc.gpsimd.affine_select(
                                s[:], s[:], pattern=[[1, QW]],
                                compare_op=ALU.is_ge, fill=0.0,
                                base=-delta, channel_multiplier=-1,
                            )
                    # yT[d, t_q] += v.T @ scoresT, col-tiled pair
                    nc.tensor.matmul(
                        ypA[0:64, :], vsb[:, kb, hp * P:hp * P + 64], sA[:],
                        start=(kb == 0), stop=(kb == kb_hi),
                        tile_position=(0, 0),
                    )
                    nc.tensor.matmul(
                        ypB[64:128, :], vsb[:, kb, hp * P + 64:hp * P + 128], sB[:],
                        start=(kb == 0), stop=(kb == kb_hi),
                        tile_position=(0, 64),
                    )
                yt = ysb.tile([P, QW], F32, tag="yt")
                nc.scalar.activation(yt[0:64, :], ypA[0:64, :], AF.Copy)
                nc.vector.tensor_copy(yt[64:128, :], ypB[64:128, :])
                nc.sync.dma_start(
                    yT[hp * P:(hp + 1) * P, qc * QW:(qc + 1) * QW], yt[:])

    nc.compile()
    return nc


_CACHE = {}


def _get_module():
    if "nc" not in _CACHE:
        _CACHE["nc"] = build_module()
    return _CACHE["nc"]


def _prep_in_maps(x, W_attn, b_attn, T=1024, C=768, n_cores=8):
    bf = ml_dtypes.bfloat16
    OT = 2 * C // P
    WT = np.ascontiguousarray(W_attn.astype(np.float32).T).astype(bf)  # [C, 3C]
    bqk = np.ascontiguousarray(
        b_attn[:2 * C].astype(np.float32).reshape(OT, P).T)            # [P, OT]
    bv = np.ascontiguousarray(
        np.tile(b_attn[2 * C:].astype(np.float32)[None, :], (P, 1)))   # [P, C]
    in_maps = []
    for c in range(n_cores):
        xT_b = np.ascontiguousarray(x[c].astype(np.float32).T).astype(bf)
        in_maps.append({"xT": xT_b, "wT": WT, "bqk": bqk, "bv": bv})
    return in_maps


def run(x, W_attn, b_attn, trace=False):
    nc = _get_module()
    in_maps = _prep_in_maps(x, W_attn, b_attn)
    res = bass_utils.run_bass_kernel_spmd(
        nc, in_maps, core_ids=list(range(8)), trace=trace)
    y = np.stack([np.asarray(res.results[c]["yT"]).T for c in range(8)])
    return np.ascontiguousarray(y.astype(np.float32)), res


def kernel(x, W_attn, b_attn):
    y, _ = run(x, W_attn, b_attn, trace=False)
    return y


# revision 8
# speedup vs baseline: 1.0737x; 1.0737x over previous
"""Causal ReLU-attention block (qkv proj + per-head attention) on 8 trn2 cores.

Sharding: pure data-parallel over batch (B=8 -> 1 batch element per core).
Per-core: x_b [T,C] -> qkv -> scoresT = k q^T (row-tiled head pairs) ->
relu/causal-mask -> yT = v.T @ scoresT (col-tiled head pairs) -> DRAM yT [C,T].
Host side: transpose/cast shards in, transpose gather out.
"""

import sys
from contextlib import ExitStack

sys.path.insert(0, "/opt/trn_rl_repo")

import ml_dtypes
import numpy as np

import concourse.bass as bass
import concourse.tile as tile
from concourse import bacc, bass_utils, mybir

P = 128
QW = 512  # t_q chunk width (PSUM bank = 512 fp32)

BF16 = mybir.dt.bfloat16
F32 = mybir.dt.float32
AF = mybir.ActivationFunctionType
ALU = mybir.AluOpType


def build_module(T=1024, C=768, H=12, n_cores=8):
    """Build + compile the per-core Bass module (same program on all cores)."""
    hd = C // H
    assert hd == 64 and H % 2 == 0 and C % P == 0 and T % QW == 0
    CT = C // P            # contraction tiles over C
    TT = T // P            # t tiles
    NQC = T // QW          # q chunks
    VW = C // 2            # v output free-dim chunk width (<=512)
    NHP = H // 2           # head pairs
    scale = 1.0 / float(np.sqrt(hd))

    nc = bacc.Bacc("TRN2", target_bir_lowering=False, debug=False,
                   num_devices=n_cores)

    xT = nc.dram_tensor("xT", [C, T], BF16, kind="ExternalInput").ap()
    wT = nc.dram_tensor("wT", [C, 3 * C], BF16, kind="ExternalInput").ap()
    bqk = nc.dram_tensor("bqk", [P, 2 * CT], F32, kind="ExternalInput").ap()
    bv = nc.dram_tensor("bv", [P, C], F32, kind="ExternalInput").ap()
    yT = nc.dram_tensor("yT", [C, T], F32, kind="ExternalOutput").ap()

    wT3 = wT.rearrange("(ct p) o -> p ct o", p=P)
    xT3 = xT.rearrange("(ct p) t -> p ct t", p=P)

    with tile.TileContext(nc) as tc, ExitStack() as ctx:
        const = ctx.enter_context(tc.tile_pool(name="const", bufs=1))
        psum = ctx.enter_context(tc.tile_pool(name="psum", bufs=3, space="PSUM"))
        ypsum = ctx.enter_context(tc.tile_pool(name="ypsum", bufs=2, space="PSUM"))
        scb = ctx.enter_context(tc.tile_pool(name="scb", bufs=6))
        ysb = ctx.enter_context(tc.tile_pool(name="ysb", bufs=3))

        # inputs staged per c-tile so the first matmuls start early
        wt_sb = const.tile([P, CT, 3 * C], BF16)
        xt_sb = const.tile([P, CT, T], BF16)
        for ct in range(CT):
            nc.sync.dma_start(wt_sb[:, ct], wT3[:, ct])
            nc.sync.dma_start(xt_sb[:, ct], xT3[:, ct])
        bqk_sb = const.tile([P, 2 * CT], F32)
        nc.sync.dma_start(bqk_sb[:], bqk[:])
        bv_sb = const.tile([P, 2, VW], F32)
        nc.sync.dma_start(bv_sb[:], bv.rearrange("p (oc v) -> p oc v", oc=2))

        qkT = const.tile([P, 2 * CT, T], BF16)   # o-tiles: q = 0..CT-1, k = CT..
        vsb = const.tile([P, TT, C], BF16)       # v in natural [t, o] layout

        evict = [0]

        def relu_evict(dst, src):
            # relu(scale * s): PSUM -> SBUF bf16, alternating ACT / DVE
            if evict[0] % 2 == 0:
                nc.scalar.activation(dst, src, AF.Relu, scale=scale)
            else:
                nc.vector.tensor_scalar(dst, src, scale, 0.0, ALU.mult, ALU.max)
            evict[0] += 1

        def emit_v(tt):
            ps = psum.tile([P, 2, QW], F32, tag="blk", name="v_ps")
            for ct in range(CT):
                for oc in range(2):
                    nc.tensor.matmul(
                        ps[:, oc, :VW],
                        xt_sb[:, ct, tt * P:(tt + 1) * P],
                        wt_sb[:, ct, 2 * C + oc * VW:2 * C + (oc + 1) * VW],
                        start=(ct == 0), stop=(ct == CT - 1),
                    )
            nc.vector.tensor_tensor(
                vsb[:, tt].rearrange("p (oc v) -> p oc v", oc=2),
                ps[:, :, :VW], bv_sb[:], ALU.add)

        def emit_qk(ot):
            ps = psum.tile([P, 2, QW], F32, tag="blk", name="qk_ps")
            for ct in range(CT):
                for qc in range(NQC):
                    nc.tensor.matmul(
                        ps[:, qc],
                        wt_sb[:, ct, ot * P:(ot + 1) * P],
                        xt_sb[:, ct, qc * QW:(qc + 1) * QW],
                        start=(ct == 0), stop=(ct == CT - 1),
                    )
            nc.scalar.activation(
                qkT[:, ot], ps[:, :NQC].rearrange("p a b -> p (a b)"),
                AF.Identity, bias=bqk_sb[:, ot:ot + 1])

        def emit_attention(hp):
            for qc in range(NQC):
                yp = ypsum.tile([P, QW], F32, tag="y")  # A -> [0:64], B -> [64:]
                kb_hi = min((qc * QW + QW - 1) // P, TT - 1)
                for kb in range(kb_hi + 1):
                    delta = max(kb * P - qc * QW, 0)   # first valid t_q col
                    w = QW - delta                     # valid width
                    sp = psum.tile([P, 2, QW], F32, tag="blk", name="s_ps")
                    for h, ppos in ((0, (0, 0)), (1, (64, 0))):
                        nc.tensor.matmul(
                            sp[:, h, delta:QW],
                            qkT[h * 64:(h + 1) * 64, CT + hp, kb * P:(kb + 1) * P],
                            qkT[h * 64:(h + 1) * 64, hp,
                                qc * QW + delta:(qc + 1) * QW],
                            start=True, stop=True, tile_position=ppos,
                        )
                    s = scb.tile([P, 2, QW], BF16, tag="s")
                    relu_evict(s[:, :, delta:QW], sp[:, :, delta:QW])
                    if kb * P > qc * QW - P:   # diagonal block: causal mask
                        nc.gpsimd.affine_select(
                            s[:, :, delta:QW], s[:, :, delta:QW],
                            pattern=[[0, 2], [1, w]],
                            compare_op=ALU.is_ge, fill=0.0,
                            base=0, channel_multiplier=-1,
                        )
                    # the two heads accumulate into disjoint partition ranges
                    # of one bank; the has_written clear is per partition row,
                    # so each head runs its own start/stop group (the sim's
                    # group checker can't see the base partition -> skip it)
                    nc.tensor.matmul(
                        yp[0:64, delta:QW], vsb[:, kb, hp * P:hp * P + 64],
                        s[:, 0, delta:QW],
                        start=(kb == 0), stop=(kb == kb_hi),
                        tile_position=(0, 0), skip_group_check=True,
                    )
                    nc.tensor.matmul(
                        yp[64:128, delta:QW],
                        vsb[:, kb, hp * P + 64:hp * P + 128],
                        s[:, 1, delta:QW],
                        start=(kb == 0), stop=(kb == kb_hi),
                        tile_position=(0, 64), skip_group_check=True,
                    )
                yt = ysb.tile([P, QW], F32, tag="yt")
                nc.scalar.activation(yt[0:64, :], yp[0:64, :], AF.Copy)
                nc.vector.tensor_copy(yt[64:128, :], yp[64:128, :])
                nc.sync.dma_start(
                    yT[hp * P:(hp + 1) * P, qc * QW:(qc + 1) * QW], yt[:])

        # v first (attention needs all of it), then per head pair: its q and k
        # o-tiles followed immediately by its attention
        for tt in range(TT):
            emit_v(tt)
        for hp in range(NHP):
            emit_qk(hp)
            emit_qk(CT + hp)
            emit_attention(hp)

    nc.compile()
    return nc


_CACHE = {}


def _get_module():
    if "nc" not in _CACHE:
        _CACHE["nc"] = build_module()
    return _CACHE["nc"]


def _prep_in_maps(x, W_attn, b_attn, T=1024, C=768, n_cores=8):
    bf = ml_dtypes.bfloat16
    OT = 2 * C // P
    WT = np.ascontiguousarray(W_attn.astype(np.float32).T).astype(bf)  # [C, 3C]
    bqk = np.ascontiguousarray(
        b_attn[:2 * C].astype(np.float32).reshape(OT, P).T)            # [P, OT]
    bv = np.ascontiguousarray(
        np.tile(b_attn[2 * C:].astype(np.float32)[None, :], (P, 1)))   # [P, C]
    in_maps = []
    for c in range(n_cores):
        xT_b = np.ascontiguousarray(x[c].astype(np.float32).T).astype(bf)
        in_maps.append({"xT": xT_b, "wT": WT, "bqk": bqk, "bv": bv})
    return in_maps


def run(x, W_attn, b_attn, trace=False):
    nc = _get_module()
    in_maps = _prep_in_maps(x, W_attn, b_attn)
    res = bass_utils.run_bass_kernel_spmd(
        nc, in_maps, core_ids=list(range(8)), trace=trace)
    y = np.stack([np.asarray(res.results[c]["yT"]).T for c in range(8)])
    return np.ascontiguousarray(y.astype(np.float32)), res


def kernel(x, W_attn, b_attn):
    y, _ = run(x, W_attn, b_attn, trace=False)
    return y


# revision 11
# speedup vs baseline: 1.1240x; 1.0469x over previous
"""Causal ReLU-attention block (qkv proj + per-head attention) on 8 trn2 cores.

Sharding: pure data-parallel over batch (B=8 -> 1 batch element per core).
Per-core: x_b [T,C] -> qkv -> scoresT = k q^T (row-tiled head pairs) ->
relu/causal-mask -> yT = v.T @ scoresT (col-tiled head pairs) -> DRAM yT [C,T].
Host side: transpose/cast shards in, transpose gather out.
"""

import sys
from contextlib import ExitStack

sys.path.insert(0, "/opt/trn_rl_repo")

import ml_dtypes
import numpy as np

import concourse.bass as bass
import concourse.tile as tile
from concourse import bacc, bass_utils, mybir

P = 128
QW = 512  # t_q chunk width (PSUM bank = 512 fp32)

BF16 = mybir.dt.bfloat16
F32 = mybir.dt.float32
AF = mybir.ActivationFunctionType
ALU = mybir.AluOpType


def build_module(T=1024, C=768, H=12, n_cores=8):
    """Build + compile the per-core Bass module (same program on all cores)."""
    hd = C // H
    assert hd == 64 and H % 2 == 0 and C % P == 0 and T % QW == 0
    CT = C // P            # contraction tiles over C
    TT = T // P            # t tiles
    NQC = T // QW          # q chunks
    VW = C // 2            # v output free-dim chunk width (<=512)
    NHP = H // 2           # head pairs
    scale = 1.0 / float(np.sqrt(hd))

    nc = bacc.Bacc("TRN2", target_bir_lowering=False, debug=False,
                   num_devices=n_cores)

    xT = nc.dram_tensor("xT", [C, T], BF16, kind="ExternalInput").ap()
    wT = nc.dram_tensor("wT", [C, 3 * C], BF16, kind="ExternalInput").ap()
    bqk = nc.dram_tensor("bqk", [P, 2 * CT], F32, kind="ExternalInput").ap()
    bv = nc.dram_tensor("bv", [P, C], F32, kind="ExternalInput").ap()
    yT = nc.dram_tensor("yT", [C, T], F32, kind="ExternalOutput").ap()

    wT3 = wT.rearrange("(ct p) o -> p ct o", p=P)
    xT3 = xT.rearrange("(ct p) t -> p ct t", p=P)

    with tile.TileContext(nc) as tc, ExitStack() as ctx:
        const = ctx.enter_context(tc.tile_pool(name="const", bufs=1))
        psum = ctx.enter_context(tc.tile_pool(name="psum", bufs=3, space="PSUM"))
        ypsum = ctx.enter_context(tc.tile_pool(name="ypsum", bufs=2, space="PSUM"))
        scb = ctx.enter_context(tc.tile_pool(name="scb", bufs=6))
        ysb = ctx.enter_context(tc.tile_pool(name="ysb", bufs=3))

        # inputs staged in fine chunks, alternating the two HWDGE queues
        # (sync / scalar), ordered by first use: o-tile-0 weight slivers and
        # the first x chunk gate the very first matmuls
        wt_sb = const.tile([P, CT, 3 * C], BF16)
        xt_sb = const.tile([P, CT, T], BF16)
        for ct in range(CT):
            nc.scalar.dma_start(wt_sb[:, ct, 0:P], wT3[:, ct, 0:P])
            nc.sync.dma_start(xt_sb[:, ct, 0:QW], xT3[:, ct, 0:QW])
        bqk_sb = const.tile([P, 2 * CT], F32)
        nc.scalar.dma_start(bqk_sb[:], bqk[:])
        for ct in range(CT):
            nc.sync.dma_start(xt_sb[:, ct, QW:T], xT3[:, ct, QW:T])
            nc.scalar.dma_start(wt_sb[:, ct, P:2 * C], wT3[:, ct, P:2 * C])
        bv_sb = const.tile([P, 2, VW], F32)
        nc.sync.dma_start(bv_sb[:], bv.rearrange("p (oc v) -> p oc v", oc=2))
        for ct in range(CT):
            eng = nc.sync if ct % 2 == 0 else nc.scalar
            eng.dma_start(wt_sb[:, ct, 2 * C:3 * C], wT3[:, ct, 2 * C:3 * C])

        qkT = const.tile([P, 2 * CT, T], BF16)   # o-tiles: q = 0..CT-1, k = CT..
        vsb = const.tile([P, TT, C], BF16)       # v in natural [t, o] layout

        evict = [0]

        def relu_evict(dst, src):
            # relu(scale * s): PSUM -> SBUF bf16, alternating ACT / DVE
            if evict[0] % 2 == 0:
                nc.scalar.activation(dst, src, AF.Relu, scale=scale)
            else:
                nc.vector.tensor_scalar(dst, src, scale, 0.0, ALU.mult, ALU.max)
            evict[0] += 1

        def emit_v(tt):
            ps = psum.tile([P, 2, QW], F32, tag="blk", name="v_ps")
            for ct in range(CT):
                for oc in range(2):
                    nc.tensor.matmul(
                        ps[:, oc, :VW],
                        xt_sb[:, ct, tt * P:(tt + 1) * P],
                        wt_sb[:, ct, 2 * C + oc * VW:2 * C + (oc + 1) * VW],
                        start=(ct == 0), stop=(ct == CT - 1),
                    )
            nc.vector.tensor_tensor(
                vsb[:, tt].rearrange("p (oc v) -> p oc v", oc=2),
                ps[:, :, :VW], bv_sb[:], ALU.add)

        def emit_qk(ot):
            # qc-outer so the first accumulation chain only needs the first
            # x chunk of each c-tile
            ps = psum.tile([P, 2, QW], F32, tag="blk", name="qk_ps")
            for qc in range(NQC):
                for ct in range(CT):
                    nc.tensor.matmul(
                        ps[:, qc],
                        wt_sb[:, ct, ot * P:(ot + 1) * P],
                        xt_sb[:, ct, qc * QW:(qc + 1) * QW],
                        start=(ct == 0), stop=(ct == CT - 1),
                    )
            nc.scalar.activation(
                qkT[:, ot], ps[:, :NQC].rearrange("p a b -> p (a b)"),
                AF.Identity, bias=bqk_sb[:, ot:ot + 1])

        def emit_attention(hp):
            for qc in range(NQC):
                yp = ypsum.tile([P, QW], F32, tag="y")  # A -> [0:64], B -> [64:]
                kb_hi = min((qc * QW + QW - 1) // P, TT - 1)
                for kb in range(kb_hi + 1):
                    delta = max(kb * P - qc * QW, 0)   # first valid t_q col
                    w = QW - delta                     # valid width
                    sp = psum.tile([P, 2, QW], F32, tag="blk", name="s_ps")
                    for h, ppos in ((0, (0, 0)), (1, (64, 0))):
                        nc.tensor.matmul(
                            sp[:, h, delta:QW],
                            qkT[h * 64:(h + 1) * 64, CT + hp, kb * P:(kb + 1) * P],
                            qkT[h * 64:(h + 1) * 64, hp,
                                qc * QW + delta:(qc + 1) * QW],
                            start=True, stop=True, tile_position=ppos,
                        )
                    s = scb.tile([P, 2, QW], BF16, tag="s")
                    relu_evict(s[:, :, delta:QW], sp[:, :, delta:QW])
                    if kb * P > qc * QW - P:   # diagonal block: causal mask
                        nc.gpsimd.affine_select(
                            s[:, :, delta:QW], s[:, :, delta:QW],
                            pattern=[[0, 2], [1, w]],
                            compare_op=ALU.is_ge, fill=0.0,
                            base=0, channel_multiplier=-1,
                        )
                    # the two heads accumulate into disjoint partition ranges
                    # of one bank; the has_written clear is per partition row,
                    # so each head runs its own start/stop group (the sim's
                    # group checker can't see the base partition -> skip it)
                    nc.tensor.matmul(
                        yp[0:64, delta:QW], vsb[:, kb, hp * P:hp * P + 64],
                        s[:, 0, delta:QW],
                        start=(kb == 0), stop=(kb == kb_hi),
                        tile_position=(0, 0), skip_group_check=True,
                    )
                    nc.tensor.matmul(
                        yp[64:128, delta:QW],
                        vsb[:, kb, hp * P + 64:hp * P + 128],
                        s[:, 1, delta:QW],
                        start=(kb == 0), stop=(kb == kb_hi),
                        tile_position=(0, 64), skip_group_check=True,
                    )
                yt = ysb.tile([P, QW], F32, tag="yt")
                nc.scalar.activation(yt[0:64, :], yp[0:64, :], AF.Copy)
                nc.vector.tensor_copy(yt[64:128, :], yp[64:128, :])
                nc.sync.dma_start(
                    yT[hp * P:(hp + 1) * P, qc * QW:(qc + 1) * QW], yt[:])

        # qk first (gated on small weight slivers), then v, then attention
        # (which needs all of v); evictions spread across the whole span
        for hp in range(NHP):
            emit_qk(hp)
            emit_qk(CT + hp)
        for tt in range(TT):
            emit_v(tt)
        for hp in range(NHP):
            emit_attention(hp)

    nc.compile()
    return nc


_CACHE = {}


def _get_module():
    if "nc" not in _CACHE:
        _CACHE["nc"] = build_module()
    return _CACHE["nc"]


def _prep_in_maps(x, W_attn, b_attn, T=1024, C=768, n_cores=8):
    bf = ml_dtypes.bfloat16
    OT = 2 * C // P
    WT = np.ascontiguousarray(W_attn.astype(np.float32).T).astype(bf)  # [C, 3C]
    bqk = np.ascontiguousarray(
        b_attn[:2 * C].astype(np.float32).reshape(OT, P).T)            # [P, OT]
    bv = np.ascontiguousarray(
        np.tile(b_attn[2 * C:].astype(np.float32)[None, :], (P, 1)))   # [P, C]
    in_maps = []
    for c in range(n_cores):
        xT_b = np.ascontiguousarray(x[c].astype(np.float32).T).astype(bf)
        in_maps.append({"xT": xT_b, "wT": WT, "bqk": bqk, "bv": bv})
    return in_maps


def run(x, W_attn, b_attn, trace=False):
    nc = _get_module()
    in_maps = _prep_in_maps(x, W_attn, b_attn)
    res = bass_utils.run_bass_kernel_spmd(
        nc, in_maps, core_ids=list(range(8)), trace=trace)
    y = np.stack([np.asarray(res.results[c]["yT"]).T for c in range(8)])
    return np.ascontiguousarray(y.astype(np.float32)), res


def kernel(x, W_attn, b_attn):
    y, _ = run(x, W_attn, b_attn, trace=False)
    return y


# revision 15
# speedup vs baseline: 1.1608x; 1.0327x over previous
"""Causal ReLU-attention block (qkv proj + per-head attention) on 8 trn2 cores.

Sharding: pure data-parallel over batch (B=8 -> 1 batch element per core).
Per-core: x_b [T,C] -> qkv -> scoresT = k q^T (row-tiled head pairs) ->
relu/causal-mask -> yT = v.T @ scoresT (col-tiled head pairs) -> DRAM yT [C,T].
Host side: transpose/cast shards in, transpose gather out.
"""

import sys
from contextlib import ExitStack

sys.path.insert(0, "/opt/trn_rl_repo")

import ml_dtypes
import numpy as np

import concourse.bass as bass
import concourse.tile as tile
from concourse import bacc, bass_utils, mybir

P = 128
QW = 512  # t_q chunk width (PSUM bank = 512 fp32)

BF16 = mybir.dt.bfloat16
F32 = mybir.dt.float32
AF = mybir.ActivationFunctionType
ALU = mybir.AluOpType


def build_module(T=1024, C=768, H=12, n_cores=8):
    """Build + compile the per-core Bass module (same program on all cores)."""
    hd = C // H
    assert hd == 64 and H % 2 == 0 and C % P == 0 and T % QW == 0
    CT = C // P            # contraction tiles over C
    TT = T // P            # t tiles
    NQC = T // QW          # q chunks
    VW = C // 2            # v output free-dim chunk width (<=512)
    NHP = H // 2           # head pairs
    scale = 1.0 / float(np.sqrt(hd))

    nc = bacc.Bacc("TRN2", target_bir_lowering=False, debug=False,
                   num_devices=n_cores)

    xT = nc.dram_tensor("xT", [C, T], BF16, kind="ExternalInput").ap()
    wT = nc.dram_tensor("wT", [C, 3 * C], BF16, kind="ExternalInput").ap()
    bqk = nc.dram_tensor("bqk", [P, 2 * CT], F32, kind="ExternalInput").ap()
    bv = nc.dram_tensor("bv", [P, C], F32, kind="ExternalInput").ap()
    yT = nc.dram_tensor("yT", [C, T], F32, kind="ExternalOutput").ap()

    wT3 = wT.rearrange("(ct p) o -> p ct o", p=P)
    xT3 = xT.rearrange("(ct p) t -> p ct t", p=P)

    with tile.TileContext(nc) as tc, ExitStack() as ctx:
        const = ctx.enter_context(tc.tile_pool(name="const", bufs=1))
        psum = ctx.enter_context(tc.tile_pool(name="psum", bufs=3, space="PSUM"))
        ypsum = ctx.enter_context(tc.tile_pool(name="ypsum", bufs=2, space="PSUM"))
        scb = ctx.enter_context(tc.tile_pool(name="scb", bufs=6))
        ysb = ctx.enter_context(tc.tile_pool(name="ysb", bufs=3))

        # inputs staged in fine chunks, alternating the two HWDGE queues
        # (sync / scalar), ordered by first use: o-tile-0 weight slivers and
        # the first x chunk gate the very first matmuls
        wt_sb = const.tile([P, CT, 3 * C], BF16)
        xt_sb = const.tile([P, CT, T], BF16)
        for ct in range(CT):
            nc.scalar.dma_start(wt_sb[:, ct, 0:P], wT3[:, ct, 0:P])
            nc.sync.dma_start(xt_sb[:, ct, 0:QW], xT3[:, ct, 0:QW])
        bqk_sb = const.tile([P, 2 * CT], F32)
        nc.scalar.dma_start(bqk_sb[:], bqk[:])
        for ct in range(CT):
            if T > QW:
                nc.sync.dma_start(xt_sb[:, ct, QW:T], xT3[:, ct, QW:T])
            nc.scalar.dma_start(wt_sb[:, ct, P:C], wT3[:, ct, P:C])
        for ct in range(CT):
            eng = nc.sync if ct % 2 == 0 else nc.scalar
            eng.dma_start(wt_sb[:, ct, C:2 * C], wT3[:, ct, C:2 * C])
        bv_sb = const.tile([P, 2, VW], F32)
        nc.sync.dma_start(bv_sb[:], bv.rearrange("p (oc v) -> p oc v", oc=2))
        for ct in range(CT):
            eng = nc.sync if ct % 2 == 0 else nc.scalar
            eng.dma_start(wt_sb[:, ct, 2 * C:3 * C], wT3[:, ct, 2 * C:3 * C])

        # 0/1 upper-triangle mask (keep j >= p) for DVE-side causal masking
        mask_sb = const.tile([P, QW], BF16)
        nc.gpsimd.memset(mask_sb[:], 1.0)
        nc.gpsimd.affine_select(
            mask_sb[:], mask_sb[:], pattern=[[1, QW]],
            compare_op=ALU.is_ge, fill=0.0, base=0, channel_multiplier=-1)

        qkT = const.tile([P, 2 * CT, T], BF16)   # o-tiles: q = 0..CT-1, k = CT..
        vsb = const.tile([P, TT, C], BF16)       # v in natural [t, o] layout

        evict = [0]

        def relu_evict(dst, src):
            # relu(scale * s): PSUM -> SBUF bf16, alternating ACT / DVE
            if evict[0] % 2 == 0:
                nc.scalar.activation(dst, src, AF.Relu, scale=scale)
            else:
                nc.vector.tensor_scalar(dst, src, scale, 0.0, ALU.mult, ALU.max)
            evict[0] += 1

        def emit_v(tt):
            ps = psum.tile([P, 2, QW], F32, tag="blk", name="v_ps")
            for ct in range(CT):
                for oc in range(2):
                    nc.tensor.matmul(
                        ps[:, oc, :VW],
                        xt_sb[:, ct, tt * P:(tt + 1) * P],
                        wt_sb[:, ct, 2 * C + oc * VW:2 * C + (oc + 1) * VW],
                        start=(ct == 0), stop=(ct == CT - 1),
                    )
            nc.vector.tensor_tensor(
                vsb[:, tt].rearrange("p (oc v) -> p oc v", oc=2),
                ps[:, :, :VW], bv_sb[:], ALU.add)

        def emit_qk(ot):
            # qc-outer so the first accumulation chain only needs the first
            # x chunk of each c-tile
            ps = psum.tile([P, 2, QW], F32, tag="blk", name="qk_ps")
            for qc in range(NQC):
                for ct in range(CT):
                    nc.tensor.matmul(
                        ps[:, qc],
                        wt_sb[:, ct, ot * P:(ot + 1) * P],
                        xt_sb[:, ct, qc * QW:(qc + 1) * QW],
                        start=(ct == 0), stop=(ct == CT - 1),
                    )
            nc.scalar.activation(
                qkT[:, ot], ps[:, :NQC].rearrange("p a b -> p (a b)"),
                AF.Identity, bias=bqk_sb[:, ot:ot + 1])

        mstep = [0]

        def attention_steps(hp):
            """Per-(qc,kb) emission closures for one head pair, so two pairs
            can be interleaved (independent work hides the evict/mask chain
            latency in front of the att@v matmuls)."""
            steps = []
            for qc in range(NQC):
                kb_hi = min((qc * QW + QW - 1) // P, TT - 1)
                state = {}

                def step(kb, qc=qc, kb_hi=kb_hi, state=state):
                    if kb == 0:
                        state["yp"] = ypsum.tile([P, QW], F32, tag="y",
                                                 name="yp")
                    yp = state["yp"]
                    delta = max(kb * P - qc * QW, 0)   # first valid t_q col
                    w = QW - delta
                    sp = psum.tile([P, 2, QW], F32, tag="blk", name="s_ps")
                    for h, ppos in ((0, (0, 0)), (1, (64, 0))):
                        nc.tensor.matmul(
                            sp[:, h, delta:QW],
                            qkT[h * 64:(h + 1) * 64, CT + hp,
                                kb * P:(kb + 1) * P],
                            qkT[h * 64:(h + 1) * 64, hp,
                                qc * QW + delta:(qc + 1) * QW],
                            start=True, stop=True, tile_position=ppos,
                        )
                    s = scb.tile([P, 2, QW], BF16, tag="s")
                    relu_evict(s[:, :, delta:QW], sp[:, :, delta:QW])
                    if kb * P > qc * QW - P:   # diagonal block: causal mask
                        if mstep[0] % 2 == 0:
                            nc.gpsimd.affine_select(
                                s[:, :, delta:QW], s[:, :, delta:QW],
                                pattern=[[0, 2], [1, w]],
                                compare_op=ALU.is_ge, fill=0.0,
                                base=0, channel_multiplier=-1,
                            )
                        else:
                            nc.vector.tensor_tensor(
                                s[:, :, delta:QW], s[:, :, delta:QW],
                                mask_sb[:, None, 0:w].to_broadcast((P, 2, w)),
                                ALU.mult,
                            )
                        mstep[0] += 1
                    # the two heads accumulate into disjoint partition ranges
                    # of one bank; each runs its own start/stop group (the
                    # sim's group checker can't see base partition -> skip)
                    nc.tensor.matmul(
                        yp[0:64, delta:QW], vsb[:, kb, hp * P:hp * P + 64],
                        s[:, 0, delta:QW],
                        start=(kb == 0), stop=(kb == kb_hi),
                        tile_position=(0, 0), skip_group_check=True,
                    )
                    nc.tensor.matmul(
                        yp[64:128, delta:QW],
                        vsb[:, kb, hp * P + 64:hp * P + 128],
                        s[:, 1, delta:QW],
                        start=(kb == 0), stop=(kb == kb_hi),
                        tile_position=(0, 64), skip_group_check=True,
                    )
                    if kb == kb_hi:
                        yt = ysb.tile([P, QW], F32, tag="yt")
                        nc.scalar.activation(yt[0:64, :], yp[0:64, :], AF.Copy)
                        nc.vector.tensor_copy(yt[64:128, :], yp[64:128, :])
                        nc.sync.dma_start(
                            yT[hp * P:(hp + 1) * P, qc * QW:(qc + 1) * QW],
                            yt[:])

                for kb in range(kb_hi + 1):
                    steps.append(lambda kb=kb, step=step: step(kb))
            return steps

        # qk first (gated on small weight slivers, in DMA arrival order),
        # then v, then attention (which needs all of v) with two head
        # pairs' chains interleaved
        for ot in range(2 * CT):
            emit_qk(ot)
        for tt in range(TT):
            emit_v(tt)
        for j in range(0, NHP - 1, 2):
            a, b = attention_steps(j), attention_steps(j + 1)
            for i in range(max(len(a), len(b))):
                if i < len(a):
                    a[i]()
                if i < len(b):
                    b[i]()
        if NHP % 2:
            for f in attention_steps(NHP - 1):
                f()

    nc.compile()
    return nc


_CACHE = {}


def _get_module():
    if "nc" not in _CACHE:
        _CACHE["nc"] = build_module()
    return _CACHE["nc"]


def _prep_in_maps(x, W_attn, b_attn, T=1024, C=768, n_cores=8):
    bf = ml_dtypes.bfloat16
    OT = 2 * C // P
    WT = np.ascontiguousarray(W_attn.astype(np.float32).T).astype(bf)  # [C, 3C]
    bqk = np.ascontiguousarray(
        b_attn[:2 * C].astype(np.float32).reshape(OT, P).T)            # [P, OT]
    bv = np.ascontiguousarray(
        np.tile(b_attn[2 * C:].astype(np.float32)[None, :], (P, 1)))   # [P, C]
    in_maps = []
    for c in range(n_cores):
        xT_b = np.ascontiguousarray(x[c].astype(np.float32).T).astype(bf)
        in_maps.append({"xT": xT_b, "wT": WT, "bqk": bqk, "bv": bv})
    return in_maps


def run(x, W_attn, b_attn, trace=False):
    nc = _get_module()
    in_maps = _prep_in_maps(x, W_attn, b_attn)
    res = bass_utils.run_bass_kernel_spmd(
        nc, in_maps, core_ids=list(range(8)), trace=trace)
    y = np.stack([np.asarray(res.results[c]["yT"]).T for c in range(8)])
    return np.ascontiguousarray(y.astype(np.float32)), res


def kernel(x, W_attn, b_attn):
    y, _ = run(x, W_attn, b_attn, trace=False)
    return y
